# revision 1
# baseline (speedup 1.0000x reference)
"""Realspace Ewald sum on 8 Trainium2 NeuronCores (Bass/Tile).

pot = NORM/(4*pi) * sum_{i!=j} q_i q_j erf(d_ij/sqrt2)/d_ij   (N=6144)

Math/implementation notes (all validated on HW piecewise):
 - S' = sigma*(d^2/2) + delta computed per [128j x 512i] tile by one PE
   matmul over K=30 features: 3-way bf16 Dekker splits of the 5 base
   features [|r|^2-ish, 1, x, y, z]; abs err ~1.5e-8 << delta=1e-6.
 - 1/d via bit-trick seed + 2 Newton steps entirely on-chip:
     seed  z0 = bitcast((bits(S')>>1) ^ 0x5FFFFFFF)  [DVE tensor_scalar]
     z2 = z1*(A2 - S'*z1^2), z1 = z0*(A1 - S'*z0^2)  [one custom 8-stage
     DVE op]; z2 = G/sqrt(2S'), rel err < 4e-6 over d in [0.004, 40].
 - u = S'*z2 (DVE stt); e = erf(ESC*u) (ACT, the only table function);
   v = e*z2 (GPSIMD, the only non-DVE elementwise engine) = G'*w.
 - weighted j-reduction on PE: acc[1, i] += q_col^T @ v, accumulated
   over a chunk of 4 j-tiles in PSUM (fp32).
 - triangular tiling: block-symmetric halving; off-diagonal tiles get
   q_col pre-doubled; diagonal tiles weight 1; the i==j element is
   regularized by delta and subtracted on the host.
 - per-chunk acc rows go to DRAM; the final q_i weighting + global sum
   happen on the host in float64.
"""

import numpy as np

import concourse.bass as bass
import concourse.bacc as bacc
import concourse.mybir as mybir
import concourse.tile as tile
from concourse.bass_utils import run_bass_kernel_spmd

# ---------------------------------------------------------------- constants
N = 6144
P = 128                     # j-tile height (partitions)
NI = 512                    # i-block width (free dim)
NJT = N // P                # 48 j-tiles
NBI = N // NI               # 12 i-blocks
NCORES = 8
TILES_PER_CHUNK = 4
BATCH = 2                   # tiles per elementwise batch
CHUNKS_TOTAL = 80           # 78 real + 2 dummy
CHUNKS_PER_CORE = CHUNKS_TOTAL // NCORES   # 10
TILES_PER_CORE = CHUNKS_PER_CORE * TILES_PER_CHUNK  # 40

SIGMA_S = 2.0 ** -9
XMASK = 0x5FFFFFFF
A1 = 26.190803617547342
A2 = 7963.254545200805
G_SCALE = 386810.75152551587
ESC = float(np.sqrt(2.0) / (G_SCALE * np.sqrt(SIGMA_S)))
WSC = float(np.sqrt(SIGMA_S) / G_SCALE)
DELTA = 1e-6

TWOPI = 2.0 * np.pi
NORM_FACTOR = 90.0474

F32 = mybir.dt.float32
BF16 = mybir.dt.bfloat16
I32 = mybir.dt.int32


# ------------------------------------------------------------ custom DVE op
def _rsqrt_nr2_reference(in0, in1, c0, c1, c2):
    f = np.float32
    z0 = in1
    z1 = (z0 * (f(c0) - in0 * z0 * z0)).astype(np.float32)
    return (z1 * (f(c1) - in0 * z1 * z1)).astype(np.float32)


def _register_rsqrt_op():
    import concourse.dve_ops as dve_ops
    from concourse.dve_ops import DveOp
    from concourse.dve_spec import Spec, Src0, Src1, C0, C1, lower
    from concourse.dve_uop import DveOpSpec

    for op in dve_ops.OPS:
        if op.name == "RSQRT_NR2_ANT":
            return op
    z0 = Src1
    z1 = z0 * (C0 - Src0 * z0 * z0)
    body = z1 * (C1 - Src0 * z1 * z1)
    spec = Spec(body=body, reference=_rsqrt_nr2_reference)
    row = dve_ops._CUSTOM_DVE_ROW_BASE + len(dve_ops.OPS)
    assert row < 0x20
    shas = {}
    for ver in ("v3", "v4"):
        try:
            uops = lower(spec, ver=ver)
            shas[ver] = DveOpSpec(
                name="RSQRT_NR2_ANT", opcode=row, uops=uops, rd1_en=True
            ).sha(ver)
        except Exception:
            pass
    op = DveOp("RSQRT_NR2_ANT", spec, subdim=False, uops_sha=shas)
    dve_ops.OPS.append(op)
    dve_ops.CUSTOM_DVE_SPECS[op.name] = op.spec
    dve_ops._SUB_OPCODE_FOR_NAME[op.name] = row
    return op


# ------------------------------------------------------------- host packing
def _split3_bf16(x):
    import ml_dtypes

    a = x.astype(ml_dtypes.bfloat16).astype(np.float64)
    r = x - a
    b = r.astype(ml_dtypes.bfloat16).astype(np.float64)
    c = (r - b).astype(ml_dtypes.bfloat16).astype(np.float64)
    return a, b, c


def _build_schedule():
    """All (bi, tj, weight) tiles of the block-triangle, grouped into
    same-bi chunks of 4, padded with 2 dummy chunks to 80 total."""
    chunks = []
    for bi in range(NBI):
        tjs = [(tj, 2.0 if tj < 4 * bi else 1.0) for tj in range(4 * bi + 4)]
        for k in range(0, len(tjs), TILES_PER_CHUNK):
            grp = tjs[k : k + TILES_PER_CHUNK]
            assert len(grp) == TILES_PER_CHUNK
            chunks.append((bi, grp))
    assert len(chunks) == 78
    # dummy chunks: bi=0, weight 0
    for _ in range(CHUNKS_TOTAL - len(chunks)):
        chunks.append((0, [(0, 0.0)] * TILES_PER_CHUNK))
    # round-robin for a touch of i-block diversity per core (not required)
    order = sorted(range(CHUNKS_TOTAL), key=lambda idx: idx % NCORES)
    return [chunks[idx] for idx in order]


def _features(q, r):
    """K=30 bf16-split feature rows. Returns (A_rows[30,N] f32-storage of
    bf16 values, B_rows[30,N])."""
    r64 = r.astype(np.float64)
    ri2 = (r64 ** 2).sum(1)
    ones = np.ones(N, np.float64)
    a_base = np.stack(
        [SIGMA_S / 2 * ri2 + DELTA, ones, r64[:, 0], r64[:, 1], r64[:, 2]], 0
    )
    b_base = np.stack(
        [
            ones,
            SIGMA_S / 2 * ri2,
            -SIGMA_S * r64[:, 0],
            -SIGMA_S * r64[:, 1],
            -SIGMA_S * r64[:, 2],
        ],
        0,
    )
    a0, a1, a2 = _split3_bf16(a_base)
    b0, b1, b2 = _split3_bf16(b_base)
    A_rows = np.concatenate([a0, a0, a1, a0, a1, a2], 0)
    B_rows = np.concatenate([b0, b1, b0, b2, b1, b0], 0)
    return A_rows, B_rows


# ------------------------------------------------------------- bass program
def _build_bass(custom_op, rep=1, v_engine="dve"):
    nc = bacc.Bacc("TRN2", target_bir_lowering=False, debug=False,
                   num_devices=NCORES)
    jf_d = nc.declare_dram_parameter(
        "jf", [TILES_PER_CORE, 30, P], BF16, isOutput=False
    )
    if_d = nc.declare_dram_parameter(
        "ifeat", [CHUNKS_PER_CORE, 30, NI], BF16, isOutput=False
    )
    qc_d = nc.declare_dram_parameter(
        "qc", [TILES_PER_CORE, P, 1], F32, isOutput=False
    )
    acc_d = nc.declare_dram_parameter(
        "acc", [CHUNKS_PER_CORE, 1, NI], F32, isOutput=True
    )
    NB = NI * BATCH  # elementwise batch width

    with tile.TileContext(nc) as tc:
        with (
            tc.tile_pool(name="jf", bufs=6) as jf_pool,
            tc.tile_pool(name="iff", bufs=2) as if_pool,
            tc.tile_pool(name="qc", bufs=6) as qc_pool,
            tc.tile_pool(name="z0", bufs=2) as z0_pool,
            tc.tile_pool(name="z2", bufs=2) as z2_pool,
            tc.tile_pool(name="u", bufs=2) as u_pool,
            tc.tile_pool(name="e", bufs=2) as e_pool,
            tc.tile_pool(name="v", bufs=2) as v_pool,
            tc.tile_pool(name="accs", bufs=2) as accs_pool,
            tc.tile_pool(name="sps", bufs=2, space="PSUM") as sp_pool,
            tc.tile_pool(name="accp", bufs=2, space="PSUM") as acc_pool,
        ):
            for c in range(CHUNKS_PER_CORE * rep):
                c = c % CHUNKS_PER_CORE
                ifeat = if_pool.tile([30, NI], BF16)
                nc.sync.dma_start(out=ifeat[:, :], in_=if_d.ap()[c])
                acc = acc_pool.tile([1, NI], F32)
                for b in range(BATCH):
                    sbig = sp_pool.tile([P, NB], F32, tag="sbig")
                    jfs, qcs = [], []
                    for k in range(TILES_PER_CHUNK // BATCH):
                        t = c * TILES_PER_CHUNK + b * (TILES_PER_CHUNK // BATCH) + k
                        jft = jf_pool.tile([30, P], BF16)
                        nc.sync.dma_start(out=jft[:, :], in_=jf_d.ap()[t])
                        qct = qc_pool.tile([P, 1], F32)
                        nc.sync.dma_start(out=qct[:, :], in_=qc_d.ap()[t])
                        jfs.append(jft)
                        qcs.append(qct)
                        nc.tensor.matmul(
                            sbig[:, k * NI : (k + 1) * NI],
                            jft[:, :],
                            ifeat[:, :],
                            start=True,
                            stop=True,
                        )
                    z0 = z0_pool.tile([P, NB], F32)
                    nc.vector.tensor_scalar(
                        out=z0[:, :].bitcast(I32),
                        in0=sbig[:, :].bitcast(I32),
                        scalar1=1,
                        scalar2=XMASK,
                        op0=mybir.AluOpType.logical_shift_right,
                        op1=mybir.AluOpType.bitwise_xor,
                    )
                    z2 = z2_pool.tile([P, NB], F32)
                    nc.vector._custom_dve(
                        custom_op,
                        out=z2[:, :],
                        in0=sbig[:, :],
                        in1=z0[:, :],
                        s0=A1,
                        s1=A2,
                        imm2=0.0,
                    )
                    u = u_pool.tile([P, NB], F32)
                    nc.vector.scalar_tensor_tensor(
                        out=u[:, :],
                        in0=sbig[:, :],
                        scalar=1.0,
                        in1=z2[:, :],
                        op0=mybir.AluOpType.mult,
                        op1=mybir.AluOpType.mult,
                    )
                    e = e_pool.tile([P, NB], F32)
                    nc.scalar.activation(
                        e[:, :],
                        u[:, :],
                        mybir.ActivationFunctionType.Erf,
                        bias=0.0,
                        scale=ESC,
                    )
                    v = v_pool.tile([P, NB], F32)
                    v_eng = {"gpsimd": nc.gpsimd, "dve": nc.vector}[v_engine]
                    v_eng.tensor_tensor(
                        out=v[:, :],
                        in0=e[:, :],
                        in1=z2[:, :],
                        op=mybir.AluOpType.mult,
                    )
                    for k in range(TILES_PER_CHUNK // BATCH):
                        tl = b * (TILES_PER_CHUNK // BATCH) + k
                        nc.tensor.matmul(
                            acc[:, :],
                            qcs[k][:, :],
                            v[:, k * NI : (k + 1) * NI],
                            start=(tl == 0),
                            stop=(tl == TILES_PER_CHUNK - 1),
                        )
                acc_sb = accs_pool.tile([1, NI], F32)
                nc.vector.tensor_copy(acc_sb[:, :], acc[:, :])
                nc.sync.dma_start(out=acc_d.ap()[c], in_=acc_sb[:, :])
    nc.compile()
    return nc


_NC_CACHE = {}


def _get_nc():
    if "nc" not in _NC_CACHE:
        op = _register_rsqrt_op()
        _NC_CACHE["nc"] = _build_bass(op)
    return _NC_CACHE["nc"]


# ------------------------------------------------------------------- kernel
def kernel(q, r, cell):
    import ml_dtypes

    q = np.asarray(q)
    r = np.asarray(r)
    qf = q.astype(np.float64).reshape(-1)
    A_rows, B_rows = _features(q, r)
    sched = _build_schedule()  # 80 chunks

    in_maps = []
    qi_rows = []  # per core: [CHUNKS_PER_CORE, NI] float64 (q_i * is-real)
    for core in range(NCORES):
        chunks = sched[core * CHUNKS_PER_CORE : (core + 1) * CHUNKS_PER_CORE]
        jf = np.zeros((TILES_PER_CORE, 30, P), np.float64)
        iff = np.zeros((CHUNKS_PER_CORE, 30, NI), np.float64)
        qc = np.zeros((TILES_PER_CORE, P, 1), np.float32)
        qir = np.zeros((CHUNKS_PER_CORE, NI), np.float64)
        for ci, (bi, grp) in enumerate(chunks):
            iff[ci] = A_rows[:, NI * bi : NI * (bi + 1)]
            any_real = False
            for k, (tj, w) in enumerate(grp):
                t = ci * TILES_PER_CHUNK + k
                jf[t] = B_rows[:, P * tj : P * (tj + 1)]
                qc[t, :, 0] = (w * qf[P * tj : P * (tj + 1)]).astype(np.float32)
                any_real = any_real or (w != 0.0)
            if any_real:
                qir[ci] = qf[NI * bi : NI * (bi + 1)]
        in_maps.append(
            {
                "jf": jf.astype(ml_dtypes.bfloat16),
                "ifeat": iff.astype(ml_dtypes.bfloat16),
                "qc": qc,
            }
        )
        qi_rows.append(qir)

    nc = _get_nc()
    res = run_bass_kernel_spmd(nc, in_maps, list(range(NCORES)))

    total = 0.0
    for core in range(NCORES):
        acc = res.results[core]["acc"].astype(np.float64).reshape(
            CHUNKS_PER_CORE, NI
        )
        total += float((acc * qi_rows[core]).sum())

    pairsum = total * WSC
    from scipy.special import erf as _erf

    d0 = float(np.sqrt(2.0 * DELTA / SIGMA_S))
    c_diag = float(_erf(d0 / np.sqrt(2.0)) / d0)
    sum_q2 = float((qf ** 2).sum())
    pot = (pairsum - c_diag * sum_q2) / TWOPI / 2.0 * NORM_FACTOR
    return np.array([pot], dtype=np.float32)


def _pack_inputs(q, r):
    """The in_maps/qi_rows packing, shared by kernel() and timed_run()."""
    import ml_dtypes

    qf = np.asarray(q).astype(np.float64).reshape(-1)
    A_rows, B_rows = _features(np.asarray(q), np.asarray(r))
    sched = _build_schedule()
    in_maps = []
    for core in range(NCORES):
        chunks = sched[core * CHUNKS_PER_CORE : (core + 1) * CHUNKS_PER_CORE]
        jf = np.zeros((TILES_PER_CORE, 30, P), np.float64)
        iff = np.zeros((CHUNKS_PER_CORE, 30, NI), np.float64)
        qc = np.zeros((TILES_PER_CORE, P, 1), np.float32)
        for ci, (bi, grp) in enumerate(chunks):
            iff[ci] = A_rows[:, NI * bi : NI * (bi + 1)]
            for k, (tj, w) in enumerate(grp):
                t = ci * TILES_PER_CHUNK + k
                jf[t] = B_rows[:, P * tj : P * (tj + 1)]
                qc[t, :, 0] = (w * qf[P * tj : P * (tj + 1)]).astype(np.float32)
        in_maps.append(
            {
                "jf": jf.astype(ml_dtypes.bfloat16),
                "ifeat": iff.astype(ml_dtypes.bfloat16),
                "qc": qc,
            }
        )
    return in_maps


def timed_run(inputs, iters=8, rep_hi=3):
    """Differential HW timing: body repeated 1x vs rep_hi x; the per-call
    dispatch overhead cancels in the difference. Returns ns per body."""
    import time

    in_maps = _pack_inputs(inputs["q"], inputs["r"])
    op = _register_rsqrt_op()
    walls = {}
    for rep in (1, rep_hi):
        nc = _build_bass(op, rep=rep)
        ts = []
        for it in range(iters + 2):
            t0 = time.perf_counter()
            run_bass_kernel_spmd(nc, in_maps, list(range(NCORES)))
            ts.append(time.perf_counter() - t0)
        walls[rep] = min(ts[2:])  # skip warmup calls
    ns = (walls[rep_hi] - walls[1]) / (rep_hi - 1) * 1e9
    globals()["_LAST_WALLS"] = walls
    return int(ns)

